# revision 6
# baseline (speedup 1.0000x reference)
"""Trainium2 Bass kernel for the coco_DAA loss (nn_DAA_66812511256800).

Math (M = N*K = 320, a = input1.reshape(M, D)):
    score = a @ a.T                                   (M, M), symmetric
    rank_X[b, c] = sum_a mask[a, c] * sig(100*(X[a, b] - X[b, c])) + 1
    out = 1 - mean(min(rank_s, rank_c) / max(rank_s, rank_c))

Key reductions used here:
  * score is symmetric, so the masked-out a == c term is exactly sig(0) = 0.5:
      score_rank = colsum + 0.5   where colsum[b,c] = sum_a sig(100*(score[b,a]-score[b,c]))
  * cider ranks only depend on block indices (cider_map repeated K times), so the
    M^3 cider reduction collapses to an N^3 = 64^3 one.

Device strategy (8 cores, SPMD, a-axis / i-axis sharded):
  * per core: PE computes score row-tiles (b on partitions) into PSUM, plus the
    per-core bias block B100[b, j] = 100*score[b, a_j] for its 40 a-values.
  * main loop: one ScalarE sigmoid per (b-tile, a): sig(-100*score[b,c] + B100[b,j])
    -> identity-matmul accumulated over a in PSUM (TensorE does the reduction).
  * cider: same trick at (64, 64) scale for the core's 8 i-values.
Host: sums the 8 partial colsums, applies the closed-form diag/+1 terms, expands
cider ranks, and reduces to the final scalar (O(M^2) numpy glue).
"""

import numpy as np
from contextlib import ExitStack

import concourse.bass as bass
import concourse.bacc as bacc
import concourse.tile as tile
from concourse import mybir
from concourse.bass_utils import run_bass_kernel_spmd
from concourse.masks import make_identity

F32 = mybir.dt.float32
AF = mybir.ActivationFunctionType

N_, K_, D_ = 64, 5, 512
M_ = N_ * K_            # 320
NCORES = 8
APC = M_ // NCORES      # 40 a-values per core
IPC = N_ // NCORES      # 8 cider rows per core
DT = 4                  # contraction chunks of 128 over D=512
BT = [(0, 128), (128, 128), (256, 64)]  # b row-tiles (start, size)

_CACHE = {}
LAST_RESULTS = None


def _build_program():
    nc = bacc.Bacc(None, target_bir_lowering=False, debug=False)
    # atx[d] = [aT d-chunk (128, 320) | per-core aT a-slice columns (128, 40)]
    atx_d = nc.dram_tensor("atx", [DT, 128, M_ + APC], F32, kind="ExternalInput").ap()
    # cmx = [cider_map (64, 64) | 100*cm.T per-core i-columns (64, 8)]
    cmx_d = nc.dram_tensor("cmx", [N_, N_ + IPC], F32, kind="ExternalInput").ap()
    colsum_d = nc.dram_tensor("colsum", [M_, M_], F32, kind="ExternalOutput").ap()
    cider_d = nc.dram_tensor("cider", [N_, N_], F32, kind="ExternalOutput").ap()

    with tile.TileContext(nc) as tc, ExitStack() as ctx:
        consts = ctx.enter_context(tc.tile_pool(name="consts", bufs=1))
        sigp = ctx.enter_context(tc.tile_pool(name="sigp", bufs=4))
        outp = ctx.enter_context(tc.tile_pool(name="outp", bufs=1))
        ps_sc = ctx.enter_context(tc.tile_pool(name="ps_sc", bufs=1, space="PSUM"))
        ps_ac = ctx.enter_context(tc.tile_pool(name="ps_ac", bufs=1, space="PSUM"))
        ps_m = ctx.enter_context(tc.tile_pool(name="ps_m", bufs=1, space="PSUM"))

        at = []
        aslt = []
        for d in range(DT):
            tx = consts.tile([128, M_ + APC], F32, tag=f"atx{d}")
            nc.sync.dma_start(out=tx, in_=atx_d[d])
            at.append(tx[:, :M_])
            aslt.append(tx[:, M_ : M_ + APC])
        cmx = consts.tile([N_, N_ + IPC], F32, tag="cmx")
        nc.sync.dma_start(out=cmx, in_=cmx_d)
        cm = cmx[:, :N_]
        cmt = cmx[:, N_ : N_ + IPC]
        ident = consts.tile([128, 128], F32, tag="ident")
        make_identity(nc, ident)

        # score row-tiles (stay in PSUM; ACT reads them as sigmoid input) and
        # per-core bias blocks B100 = 100 * score[:, a_range]
        score_ps = []
        b100 = []
        for ti, (b0, bs) in enumerate(BT):
            sp = ps_sc.tile([128, M_], F32, tag=f"score{ti}")
            for d in range(DT):
                nc.tensor.matmul(
                    sp[:bs], at[d][:, b0 : b0 + bs], at[d][:, :],
                    start=(d == 0), stop=(d == DT - 1),
                )
            bp = ps_m.tile([128, APC], F32, tag="b100ps")
            for d in range(DT):
                nc.tensor.matmul(
                    bp[:bs], at[d][:, b0 : b0 + bs], aslt[d][:, :],
                    start=(d == 0), stop=(d == DT - 1),
                )
            bsb = consts.tile([128, APC], F32, tag=f"b100sb{ti}")
            nc.scalar.mul(bsb[:bs], bp[:bs], 100.0)
            score_ps.append(sp)
            b100.append(bsb)

        # main loop: sigmoid + accumulate over the core's 40 a-values
        for ti, (b0, bs) in enumerate(BT):
            ac = ps_ac.tile([128, M_], F32, tag=f"acc{ti}")
            for j in range(APC):
                sg = sigp.tile([128, M_], F32, tag="sig")
                nc.scalar.activation(
                    sg[:bs], score_ps[ti][:bs], AF.Sigmoid,
                    bias=b100[ti][:bs, j : j + 1], scale=-100.0,
                )
                nc.tensor.matmul(
                    ac[:bs], ident[:bs, :bs], sg[:bs],
                    start=(j == 0), stop=(j == APC - 1),
                )
            ob = outp.tile([128, M_], F32, tag=f"out{ti}")
            nc.vector.tensor_copy(ob[:bs], ac[:bs])
            nc.sync.dma_start(out=colsum_d[b0 : b0 + bs, :], in_=ob[:bs])

        # cider partial: sum_i sig(100*(cm[i, p] - cm[p, q])) over the core's i's
        cacc = ps_m.tile([64, N_], F32, tag="cidacc")
        for j in range(IPC):
            cs = sigp.tile([64, N_], F32, tag="csig")
            nc.scalar.activation(
                cs, cm, AF.Sigmoid, bias=cmt[:, j : j + 1], scale=-100.0
            )
            nc.tensor.matmul(
                cacc, ident[:64, :64], cs, start=(j == 0), stop=(j == IPC - 1)
            )
        cob = outp.tile([64, N_], F32, tag="outc")
        nc.vector.tensor_copy(cob, cacc)
        nc.sync.dma_start(out=cider_d, in_=cob)

    nc.compile()
    return nc


def _tsig64(x):
    # faithful f64 version of the reference's clipped temperature sigmoid
    e = np.clip(-x / 0.01, -50.0, 50.0)
    return 1.0 / (1.0 + np.exp(e))


def kernel(input1, input2, cider_map):
    global LAST_RESULTS
    if "nc" not in _CACHE:
        _CACHE["nc"] = _build_program()
    nc = _CACHE["nc"]

    a = np.ascontiguousarray(np.asarray(input1, dtype=np.float32).reshape(M_, D_))
    at4 = np.ascontiguousarray(a.T).reshape(DT, 128, M_)
    cm = np.ascontiguousarray(np.asarray(cider_map, dtype=np.float32))
    cmt100 = np.ascontiguousarray(100.0 * cm.T.astype(np.float32))

    in_maps = []
    for c in range(NCORES):
        atx = np.concatenate(
            [at4, at4[:, :, c * APC : (c + 1) * APC]], axis=2
        ).astype(np.float32)
        cmx = np.concatenate(
            [cm, cmt100[:, c * IPC : (c + 1) * IPC]], axis=1
        ).astype(np.float32)
        in_maps.append({"atx": np.ascontiguousarray(atx), "cmx": np.ascontiguousarray(cmx)})

    LAST_RESULTS = run_bass_kernel_spmd(nc, in_maps, core_ids=list(range(NCORES)))
    res = LAST_RESULTS.results

    colsum = np.zeros((M_, M_), dtype=np.float64)
    s_cm = np.zeros((N_, N_), dtype=np.float64)
    for r in res:
        colsum += r["colsum"].astype(np.float64)
        s_cm += r["cider"].astype(np.float64)

    score_rank = colsum + 0.5  # -sig(0) + 1
    cmf = cm.astype(np.float64)
    t2 = _tsig64(cmf.T - cmf)  # [p, q] -> tsig(cm[q, p] - cm[p, q])
    cider_rank_n = K_ * s_cm - t2 + 1.0
    cider_rank = np.repeat(np.repeat(cider_rank_n, K_, axis=0), K_, axis=1)

    mn = np.minimum(cider_rank, score_rank)
    mx = np.maximum(cider_rank, score_rank)
    asp = (mn / mx).mean()
    return np.float32(1.0 - asp)


# revision 7
# speedup vs baseline: 1.6508x; 1.6508x over previous
"""Trainium2 Bass kernel for the coco_DAA loss (nn_DAA_66812511256800).

Math (M = N*K = 320, a = input1.reshape(M, D)):
    score = a @ a.T                                   (M, M), symmetric
    rank_X[b, c] = sum_a mask[a, c] * sig(100*(X[a, b] - X[b, c])) + 1
    out = 1 - mean(min(rank_s, rank_c) / max(rank_s, rank_c))

Key reductions:
  * score is symmetric -> the masked diag term is exactly sig(0) = 0.5:
      score_rank = colsum + 0.5,  colsum[b,c] = sum_a sig(100*(score[b,a]-score[b,c]))
  * cider ranks only depend on block indices (cider_map repeated K times), so the
    M^3 cider reduction collapses to an N^3 = 64^3 one.

Device strategy (8 cores SPMD, reduction axes sharded):
  * PE: score row-tiles via aT-chunk matmuls; the moving operand is extended with
    the per-core 40 aT columns so the per-core bias block B100 = 100*score[:,a_rng]
    falls out of the same matmuls. The 64-row tail tile is computed twice into
    both partition halves so every later instruction runs 128 partitions wide.
  * ScalarE mul x100 -> S100X tiles in SBUF (score || bias block).
  * VectorE: z[b, c] = S100X[b,c] - S100X[b, 320+j]  (tensor_scalar, per a)
  * ScalarE: sigmoid over G*320-wide groups (amortizes the per-instr bubble),
    fp16 output.
  * PE: identity-matmul accumulation of sig tiles over a into PSUM (f32).
  * cider: same bias trick at (64,64): 8 sigmoids + 8 identity matmuls.
Host: sums partials over cores, applies closed-form diag/+1 terms, expands cider
ranks, reduces to the scalar. All O(M^2) numpy glue.
"""

import numpy as np
from contextlib import ExitStack

import concourse.bass as bass
import concourse.bacc as bacc
import concourse.tile as tile
from concourse import mybir
from concourse.bass_utils import run_bass_kernel_spmd
from concourse.masks import make_identity

F32 = mybir.dt.float32
F16 = mybir.dt.float16
AF = mybir.ActivationFunctionType

N_, K_, D_ = 64, 5, 512
M_ = N_ * K_            # 320
NCORES = 8
APC = M_ // NCORES      # 40 a-values per core
IPC = N_ // NCORES      # 8 cider rows per core
DT = 4                  # contraction chunks of 128 over D=512
MX = M_ + APC           # 360: score row || bias block
G = 10                  # sigmoid group size (a-values per ACT instruction)

_CACHE = {}
LAST_RESULTS = None


def _build_program():
    nc = bacc.Bacc(None, target_bir_lowering=False, debug=False)
    # atx[d] = [aT d-chunk (128, 320) | per-core aT a-slice columns (128, 40)]
    atx_d = nc.dram_tensor("atx", [DT, 128, MX], F32, kind="ExternalInput").ap()
    # cmx = [cider_map (64, 64) | 100*cm.T per-core i-columns (64, 8)]
    cmx_d = nc.dram_tensor("cmx", [N_, N_ + IPC], F32, kind="ExternalInput").ap()
    # rows 0:128 tile0, 128:256 tile1, 256:384 tail tile (two half-sums)
    colsum_d = nc.dram_tensor("colsum", [M_ + 64, M_], F32, kind="ExternalOutput").ap()
    cider_d = nc.dram_tensor("cider", [N_, N_], F32, kind="ExternalOutput").ap()

    with tile.TileContext(nc) as tc, ExitStack() as ctx:
        consts = ctx.enter_context(tc.tile_pool(name="consts", bufs=1))
        zpool = ctx.enter_context(tc.tile_pool(name="zpool", bufs=3))
        sigpool = ctx.enter_context(tc.tile_pool(name="sigpool", bufs=3))
        outp = ctx.enter_context(tc.tile_pool(name="outp", bufs=1))
        ps_w = ctx.enter_context(tc.tile_pool(name="ps_w", bufs=2, space="PSUM"))
        ps_ac = ctx.enter_context(tc.tile_pool(name="ps_ac", bufs=1, space="PSUM"))

        at = []
        for d in range(DT):
            tx = consts.tile([128, MX], F32, tag=f"atx{d}")
            nc.sync.dma_start(out=tx, in_=atx_d[d])
            at.append(tx)
        cmx = consts.tile([N_, N_ + IPC], F32, tag="cmx")
        nc.sync.dma_start(out=cmx, in_=cmx_d)
        cm = cmx[:, :N_]
        cmt = cmx[:, N_ : N_ + IPC]
        ident = consts.tile([128, 128], F16, tag="ident")
        make_identity(nc, ident)

        # S100X tiles: 100*(score row-tile || per-core bias block), f32 in SBUF.
        # Tail tile (64 rows) is materialized in both partition halves.
        s100x = []
        for ti in range(3):
            sp = ps_w.tile([128, MX], F32, tag="scoreps")
            if ti < 2:
                b0 = 128 * ti
                for d in range(DT):
                    nc.tensor.matmul(
                        sp, at[d][:, b0 : b0 + 128], at[d][:, :],
                        start=(d == 0), stop=(d == DT - 1),
                    )
            else:
                for half in range(2):
                    for d in range(DT):
                        nc.tensor.matmul(
                            sp[64 * half : 64 * half + 64, :],
                            at[d][:, 256:320], at[d][:, :],
                            start=(d == 0), stop=(d == DT - 1),
                        )
            sx = consts.tile([128, MX], F32, tag=f"s100x{ti}")
            nc.scalar.mul(sx, sp, 100.0)
            s100x.append(sx)

        # bias columns for the duplicated tail tile: top half uses j 0..19,
        # bottom half uses j 20..39 (SBUF->SBUF DMA moves across partitions)
        bdup = consts.tile([128, APC // 2], F32, tag="bdup")
        nc.sync.dma_start(out=bdup[0:64, :], in_=s100x[2][0:64, M_ : M_ + 20])
        nc.sync.dma_start(out=bdup[64:128, :], in_=s100x[2][64:128, M_ + 20 : MX])

        # main loop: z on DVE, grouped sigmoid on ACT, accumulate on PE
        njs = [APC, APC, APC // 2]
        for ti in range(3):
            ac = ps_ac.tile([128, M_], F32, tag=f"acc{ti}")
            nj = njs[ti]
            for g0 in range(0, nj, G):
                gn = min(G, nj - g0)
                zb = zpool.tile([128, G * M_], F32, tag="z")
                for k in range(gn):
                    j = g0 + k
                    col = (
                        s100x[ti][:, M_ + j : M_ + j + 1]
                        if ti < 2
                        else bdup[:, j : j + 1]
                    )
                    nc.vector.tensor_scalar_sub(
                        zb[:, k * M_ : (k + 1) * M_], s100x[ti][:, :M_], col
                    )
                sg = sigpool.tile([128, G * M_], F16, tag="sig")
                nc.scalar.activation(
                    sg[:, : gn * M_], zb[:, : gn * M_], AF.Sigmoid, scale=-1.0
                )
                for k in range(gn):
                    j = g0 + k
                    nc.tensor.matmul(
                        ac, ident, sg[:, k * M_ : (k + 1) * M_],
                        start=(j == 0), stop=(j == nj - 1),
                    )
            ob = outp.tile([128, M_], F32, tag=f"out{ti}")
            nc.vector.tensor_copy(ob, ac)
            nc.sync.dma_start(out=colsum_d[128 * ti : 128 * ti + 128, :], in_=ob)

        # cider partial: sum_i sig(100*(cm[i, p] - cm[p, q])) over the core's i's
        cacc = ps_w.tile([64, N_], F32, tag="cidacc")
        for j in range(IPC):
            cs = sigpool.tile([64, N_], F16, tag="csig")
            nc.scalar.activation(
                cs, cm, AF.Sigmoid, bias=cmt[:, j : j + 1], scale=-100.0
            )
            nc.tensor.matmul(
                cacc, ident[:64, :64], cs, start=(j == 0), stop=(j == IPC - 1)
            )
        cob = outp.tile([64, N_], F32, tag="outc")
        nc.vector.tensor_copy(cob, cacc)
        nc.sync.dma_start(out=cider_d, in_=cob)

    nc.compile()
    return nc


def _tsig64(x):
    # faithful f64 version of the reference's clipped temperature sigmoid
    e = np.clip(-x / 0.01, -50.0, 50.0)
    return 1.0 / (1.0 + np.exp(e))


def kernel(input1, input2, cider_map):
    global LAST_RESULTS
    if "nc" not in _CACHE:
        _CACHE["nc"] = _build_program()
    nc = _CACHE["nc"]

    a = np.ascontiguousarray(np.asarray(input1, dtype=np.float32).reshape(M_, D_))
    at4 = np.ascontiguousarray(a.T).reshape(DT, 128, M_)
    cm = np.ascontiguousarray(np.asarray(cider_map, dtype=np.float32))
    cmt100 = np.ascontiguousarray(100.0 * cm.T.astype(np.float32))

    in_maps = []
    for c in range(NCORES):
        atx = np.concatenate(
            [at4, at4[:, :, c * APC : (c + 1) * APC]], axis=2
        ).astype(np.float32)
        cmx = np.concatenate(
            [cm, cmt100[:, c * IPC : (c + 1) * IPC]], axis=1
        ).astype(np.float32)
        in_maps.append(
            {"atx": np.ascontiguousarray(atx), "cmx": np.ascontiguousarray(cmx)}
        )

    LAST_RESULTS = run_bass_kernel_spmd(nc, in_maps, core_ids=list(range(NCORES)))
    res = LAST_RESULTS.results

    colsum = np.zeros((M_, M_), dtype=np.float64)
    s_cm = np.zeros((N_, N_), dtype=np.float64)
    for r in res:
        cs = r["colsum"].astype(np.float64)
        colsum[:256] += cs[:256]
        colsum[256:] += cs[256:320] + cs[320:384]
        s_cm += r["cider"].astype(np.float64)

    score_rank = colsum + 0.5  # -sig(0) + 1
    cmf = cm.astype(np.float64)
    t2 = _tsig64(cmf.T - cmf)  # [p, q] -> tsig(cm[q, p] - cm[p, q])
    cider_rank_n = K_ * s_cm - t2 + 1.0
    cider_rank = np.repeat(np.repeat(cider_rank_n, K_, axis=0), K_, axis=1)

    mn = np.minimum(cider_rank, score_rank)
    mx = np.maximum(cider_rank, score_rank)
    asp = (mn / mx).mean()
    return np.float32(1.0 - asp)
